# revision 44
# baseline (speedup 1.0000x reference)
"""Trainium2 Bass kernel for nn_Conv2d: x[32,128,56,56] * W[256,128,3,3] + b -> [32,256,56,56].

Stride 1, padding 1, dilation 1. Data-parallel over batch across 8 NeuronCores
(4 images per core, no collectives).

Per core: 1D Winograd F(2,3) along W. The host transforms the padded input
into 4 components per output-column pair (v0=d0-d2, v1=d1+d2, v2=d2-d1,
v3=d1-d3) and the weights into matching components per vertical tap
(g0=w0, g1=(w0+w1+w2)/2, g2=(w0-w1+w2)/2, g3=w2). On device, each
(row_tile, cout_chunk, image_pair) group runs 12 bf16 matmuls (4 components
x 3 vertical taps, accumulated over taps) into 4 PSUM banks of
[128cout, 2img*8rows*28pairs=448], i.e. 12*448 PE rows per 896 outputs vs
18*448 for direct conv -- a 1.5x tensor-engine reduction. The inverse
transform + bias (y_even = m0+m1+m2+b, y_odd = m1-m2-m3+b) is split across
the scalar (activation with bias/scale), vector, and gpsimd engines and
writes the interleaved output columns, fully hidden under the matmuls.

Self-contained: hardcodes shapes; host does padding/Winograd/bf16 prep so
every device DMA is contiguous.
"""

import numpy as np

B, CIN, H, W_ = 32, 128, 56, 56
COUT, KH, KW = 256, 3, 3
NCORES = 8
BPC = B // NCORES          # images per core
R = 8                      # output rows per tile
NT = H // R                # row tiles per image
HP = H + 2                 # padded rows
J = W_ // 2                # output column pairs
NCOMP = 4                  # Winograd F(2,3) components
NCH = COUT // 128          # cout chunks
NP = BPC // 2              # image pairs per group (matmul free dim 2*R*J=448)

_cache = {}


def _build():
    import concourse.mybir as mybir
    import concourse.tile as tile
    from concourse import bacc

    dt = mybir.dt

    nc = bacc.Bacc("TRN2", target_bir_lowering=False, debug=False)

    # Host-transformed input per row-tile: padded rows ht*R..ht*R+R+1,
    # 4 Winograd components x 28 column pairs. Halo rows are duplicated
    # host-side so every row-tile's DMA is self-contained (a halo-free
    # layout saved 1.4MB but its chunk dependencies stalled the early
    # groups behind ~2.4MB of queued input -- measured slower).
    v_d = nc.dram_tensor(
        "v", [NT, CIN, BPC, R + 2, NCOMP, J], dt.bfloat16, kind="ExternalInput"
    )
    # Host-transformed weights: [chunk, comp, cin, kh, cout_slice] -- comp-major
    # so the first accumulation group's weights arrive in one small DMA.
    wt_d = nc.dram_tensor(
        "wt", [NCH, NCOMP, CIN, KH, 128], dt.bfloat16, kind="ExternalInput"
    )
    b_d = nc.dram_tensor("bias", [128, NCH], dt.float32, kind="ExternalInput")
    # fp16 output halves the dominant DMA term (12.8MB -> 6.4MB per core);
    # the host converts back to fp32. Quantization adds ~5e-4 absmax error.
    o_d = nc.dram_tensor("out", [BPC, COUT, H, W_], dt.float16, kind="ExternalOutput")

    with tile.TileContext(nc) as tc:
        with (
            tc.tile_pool(name="const", bufs=1) as const_pool,
            tc.tile_pool(name="vin", bufs=1) as vin_pool,
            tc.tile_pool(name="tmp", bufs=8) as tmp_pool,
            tc.tile_pool(name="outp", bufs=8) as out_pool,
            tc.tile_pool(name="psum", bufs=8, space="PSUM") as psum_pool,
        ):
            # All NT row-tiles stay resident (~63KB/partition at bf16).
            vt = {}

            # Issue order tracks the first group's critical path: weights
            # chunk 0 and row-tile 0 first, then the rest of the inputs.
            w_t = const_pool.tile([CIN, NCH, NCOMP, KH, 128], dt.bfloat16)
            for comp in range(NCOMP):
                nc.sync.dma_start(w_t[:, 0, comp], wt_d[0, comp])
            t0_ = vin_pool.tile([CIN, BPC, R + 2, NCOMP, J], dt.bfloat16, tag="v0")
            vt[0] = t0_
            for n in range(BPC):
                nc.sync.dma_start(t0_[:, n], v_d[0, :, n])
            for comp in range(NCOMP):
                nc.sync.dma_start(w_t[:, 1, comp], wt_d[1, comp])
            b_t = const_pool.tile([128, NCH], dt.float32)
            nc.sync.dma_start(b_t[:], b_d[:])
            for ht in range(1, NT):
                t_ = vin_pool.tile(
                    [CIN, BPC, R + 2, NCOMP, J], dt.bfloat16, tag=f"v{ht}", name="v"
                )
                vt[ht] = t_
                for n in range(BPC):
                    nc.sync.dma_start(t_[:, n], v_d[ht, :, n])

            # Output staging: per (c, image-pair) buffer holding up to TWO
            # row-tiles (16 rows) in fp16, flushed as one DMA per image with
            # 1792B per-partition descriptors. The odd row-tile (NT=7) is
            # flushed mid-run at ht=4 so the kernel-tail flush keeps the big
            # descriptors: segments (0,1), (2,3), (4), (5,6).
            SEG = {0: (0, 2), 1: (0, 2), 2: (2, 2), 3: (2, 2),
                   4: (4, 2), 5: (4, 2), 6: (6, 1)}
            ot_buf = {}

            for ht in range(NT):
                for c in range(NCH):
                    for p in range(NP):
                        ps = [
                            psum_pool.tile(
                                [128, 2, R, J], dt.float32, tag="ps", name="ps"
                            )
                            for _ in range(NCOMP)
                        ]
                        def mm_group(comp):
                            for kh in range(KH):
                                nc.tensor.matmul(
                                    ps[comp][:],
                                    w_t[:, c, comp, kh],
                                    vt[ht][
                                        :, 2 * p : 2 * p + 2, kh : kh + R, comp
                                    ],
                                    start=(kh == 0),
                                    stop=(kh == KH - 1),
                                )

                        # Inverse transform + bias in 6 ops (scalar 2, vector 2,
                        # gpsimd 2); every op reads at most one PSUM operand and
                        # gpsimd (no PSUM access) gets the SBUF-only finals:
                        #   y_even = ((m1 + b) + m0) + m2
                        #   y_odd  = ((m1 + b) - m3) - m2
                        # Drains are interleaved between accumulation groups so
                        # PSUM banks recycle early and the consumer engines
                        # start mid-group instead of at group end.
                        mm_group(0)
                        mm_group(1)
                        s1b = tmp_pool.tile([128, 2, R, J], dt.float32, tag="s1b")
                        nc.scalar.activation(
                            s1b[:],
                            ps[1][:],
                            mybir.ActivationFunctionType.Identity,
                            bias=b_t[:, c : c + 1],
                        )
                        mm_group(2)
                        s2 = tmp_pool.tile([128, 2, R, J], dt.float32, tag="s2")
                        nc.scalar.activation(
                            s2[:], ps[2][:], mybir.ActivationFunctionType.Identity
                        )
                        ae = tmp_pool.tile([128, 2, R, J], dt.float32, tag="ae")
                        nc.vector.tensor_add(ae[:], s1b[:], ps[0][:])
                        mm_group(3)
                        ao = tmp_pool.tile([128, 2, R, J], dt.float32, tag="ao")
                        nc.vector.tensor_sub(ao[:], s1b[:], ps[3][:])
                        # gpsimd cannot access PSUM: it gets the SBUF-only finals
                        seg0, seglen = SEG[ht]
                        if ht == seg0:
                            ot_buf[(c, p)] = out_pool.tile(
                                [128, 2, 2, R, W_], dt.float16, tag="ot", name="ot"
                            )
                        ot = ot_buf[(c, p)]
                        par = ht - seg0
                        if ht == NT - 1:
                            # Tail: half-row finals + flushes so the first
                            # output DMA starts before the second half's
                            # elementwise ops finish, and the last exposed
                            # DMA is only 4 rows.
                            for h2 in range(2):
                                rs = slice(h2 * 4, h2 * 4 + 4)
                                nc.gpsimd.tensor_add(
                                    ot[:, :, par, rs, 0::2],
                                    ae[:, :, rs],
                                    s2[:, :, rs],
                                )
                                nc.gpsimd.tensor_sub(
                                    ot[:, :, par, rs, 1::2],
                                    ao[:, :, rs],
                                    s2[:, :, rs],
                                )
                                h0 = ht * R + h2 * 4
                                for i in range(2):
                                    nc.sync.dma_start(
                                        o_d[
                                            2 * p + i,
                                            c * 128 : (c + 1) * 128,
                                            h0 : h0 + 4,
                                            :,
                                        ],
                                        ot[:, i, par, rs],
                                    )
                        else:
                            nc.gpsimd.tensor_add(
                                ot[:, :, par, :, 0::2], ae[:], s2[:]
                            )
                            nc.gpsimd.tensor_sub(
                                ot[:, :, par, :, 1::2], ao[:], s2[:]
                            )
                            if par == seglen - 1:
                                h0 = seg0 * R
                                nrows = seglen * R
                                for i in range(2):
                                    nc.sync.dma_start(
                                        o_d[
                                            2 * p + i,
                                            c * 128 : (c + 1) * 128,
                                            h0 : h0 + nrows,
                                            :,
                                        ],
                                        ot[:, i, : par + 1],
                                    )

    nc.compile()
    return nc


def _make_in_maps(x, W, b):
    import ml_dtypes

    bf16 = ml_dtypes.bfloat16
    x = np.asarray(x, dtype=np.float32)
    W = np.asarray(W, dtype=np.float32)
    b = np.asarray(b, dtype=np.float32)

    # Pad, then 1D Winograd F(2,3) input transform along W (on padded cols):
    # output pair j uses padded cols 2j..2j+3.
    xpad = np.zeros((B, CIN, HP, W_ + 2), dtype=np.float32)
    xpad[:, :, 1 : H + 1, 1 : W_ + 1] = x
    e = xpad[..., 0::2]  # even padded cols 0,2,..,56 (29)
    o = xpad[..., 1::2]  # odd padded cols 1,3,..,57 (29)
    V = np.empty((B, CIN, HP, NCOMP, J), dtype=np.float32)
    V[:, :, :, 0] = e[..., :J] - e[..., 1 : J + 1]   # d0-d2
    V[:, :, :, 1] = o[..., :J] + e[..., 1 : J + 1]   # d1+d2
    V[:, :, :, 2] = e[..., 1 : J + 1] - o[..., :J]   # d2-d1
    V[:, :, :, 3] = o[..., :J] - o[..., 1 : J + 1]   # d1-d3

    # Re-tile: [B, CIN, HP, 4, J] -> [NT, CIN, B, R+2, 4, J]
    vtiles = np.empty((NT, CIN, B, R + 2, NCOMP, J), dtype=bf16)
    for ht in range(NT):
        vtiles[ht] = V[:, :, ht * R : ht * R + R + 2].transpose(1, 0, 2, 3, 4)

    # Weight transform: per kh tap, comps [w0, (w0+w1+w2)/2, (w0-w1+w2)/2, w2]
    w0, w1, w2 = W[..., 0], W[..., 1], W[..., 2]  # each [COUT, CIN, KH]
    g = np.stack(
        [w0, (w0 + w1 + w2) * 0.5, (w0 - w1 + w2) * 0.5, w2], axis=-1
    )  # [COUT, CIN, KH, 4]
    # -> [chunk, comp, cin, kh, cout_slice]
    wt = np.ascontiguousarray(
        g.reshape(NCH, 128, CIN, KH, NCOMP).transpose(0, 4, 2, 3, 1), dtype=bf16
    )
    bh = np.ascontiguousarray(b.reshape(NCH, 128).T)

    return [
        {
            "v": np.ascontiguousarray(vtiles[:, :, core * BPC : (core + 1) * BPC]),
            "wt": wt,
            "bias": bh,
        }
        for core in range(NCORES)
    ]


def kernel(x, W, b):
    from concourse.bass_utils import run_bass_kernel_spmd

    if "nc" not in _cache:
        _cache["nc"] = _build()
    nc = _cache["nc"]

    in_maps = _make_in_maps(x, W, b)
    try:
        res = run_bass_kernel_spmd(nc, in_maps, list(range(NCORES))).results
    except Exception:
        # A prior session can leave the accelerator in a transient
        # unrecoverable state; one retry after re-init clears it.
        import time

        time.sleep(15)
        res = run_bass_kernel_spmd(nc, in_maps, list(range(NCORES))).results
    return np.concatenate(
        [res[i]["out"].astype(np.float32) for i in range(NCORES)], axis=0
    )


# revision 45
# speedup vs baseline: 1.0144x; 1.0144x over previous
"""Trainium2 Bass kernel for nn_Conv2d: x[32,128,56,56] * W[256,128,3,3] + b -> [32,256,56,56].

Stride 1, padding 1, dilation 1. Data-parallel over batch across 8 NeuronCores
(4 images per core, no collectives).

Per core: 1D Winograd F(2,3) along W. The host transforms the padded input
into 4 components per output-column pair (v0=d0-d2, v1=d1+d2, v2=d2-d1,
v3=d1-d3) and the weights into matching components per vertical tap
(g0=w0, g1=(w0+w1+w2)/2, g2=(w0-w1+w2)/2, g3=w2). On device, each
(row_tile, cout_chunk, image_pair) group runs 12 bf16 matmuls (4 components
x 3 vertical taps, accumulated over taps) into 4 PSUM banks of
[128cout, 2img*8rows*28pairs=448], i.e. 12*448 PE rows per 896 outputs vs
18*448 for direct conv -- a 1.5x tensor-engine reduction. The inverse
transform + bias (y_even = m0+m1+m2+b, y_odd = m1-m2-m3+b) is split across
the scalar (activation with bias/scale), vector, and gpsimd engines and
writes the interleaved output columns, fully hidden under the matmuls.

Self-contained: hardcodes shapes; host does padding/Winograd/bf16 prep so
every device DMA is contiguous.
"""

import numpy as np

B, CIN, H, W_ = 32, 128, 56, 56
COUT, KH, KW = 256, 3, 3
NCORES = 8
BPC = B // NCORES          # images per core
R = 8                      # output rows per tile
NT = H // R                # row tiles per image
HP = H + 2                 # padded rows
J = W_ // 2                # output column pairs
NCOMP = 4                  # Winograd F(2,3) components
NCH = COUT // 128          # cout chunks
NP = BPC // 2              # image pairs per group (matmul free dim 2*R*J=448)

_cache = {}


def _build():
    import concourse.mybir as mybir
    import concourse.tile as tile
    from concourse import bacc

    dt = mybir.dt

    nc = bacc.Bacc("TRN2", target_bir_lowering=False, debug=False)

    # Host-transformed input per row-tile: padded rows ht*R..ht*R+R+1,
    # 4 Winograd components x 28 column pairs. Halo rows are duplicated
    # host-side so every row-tile's DMA is self-contained (a halo-free
    # layout saved 1.4MB but its chunk dependencies stalled the early
    # groups behind ~2.4MB of queued input -- measured slower).
    v_d = nc.dram_tensor(
        "v", [NT, CIN, BPC, R + 2, NCOMP, J], dt.bfloat16, kind="ExternalInput"
    )
    # Host-transformed weights: [chunk, comp, cin, kh, cout_slice] -- comp-major
    # so the first accumulation group's weights arrive in one small DMA.
    wt_d = nc.dram_tensor(
        "wt", [NCH, NCOMP, CIN, KH, 128], dt.bfloat16, kind="ExternalInput"
    )
    b_d = nc.dram_tensor("bias", [128, NCH], dt.float32, kind="ExternalInput")
    # fp16 output halves the dominant DMA term (12.8MB -> 6.4MB per core);
    # the host converts back to fp32. Quantization adds ~5e-4 absmax error.
    o_d = nc.dram_tensor("out", [BPC, COUT, H, W_], dt.float16, kind="ExternalOutput")

    with tile.TileContext(nc) as tc:
        with (
            tc.tile_pool(name="const", bufs=1) as const_pool,
            tc.tile_pool(name="vin", bufs=1) as vin_pool,
            tc.tile_pool(name="tmp", bufs=8) as tmp_pool,
            tc.tile_pool(name="outp", bufs=8) as out_pool,
            tc.tile_pool(name="psum", bufs=8, space="PSUM") as psum_pool,
        ):
            # All NT row-tiles stay resident (~63KB/partition at bf16).
            vt = {}

            # Issue order tracks the first group's critical path: weights
            # chunk 0 and row-tile 0 first, then the rest of the inputs.
            w_t = const_pool.tile([CIN, NCH, NCOMP, KH, 128], dt.bfloat16)
            for comp in range(NCOMP):
                nc.sync.dma_start(w_t[:, 0, comp], wt_d[0, comp])
            t0_ = vin_pool.tile([CIN, BPC, R + 2, NCOMP, J], dt.bfloat16, tag="v0")
            vt[0] = t0_
            for n in range(BPC):
                nc.sync.dma_start(t0_[:, n], v_d[0, :, n])
            for comp in range(NCOMP):
                nc.sync.dma_start(w_t[:, 1, comp], wt_d[1, comp])
            b_t = const_pool.tile([128, NCH], dt.float32)
            nc.sync.dma_start(b_t[:], b_d[:])
            for ht in range(1, NT):
                t_ = vin_pool.tile(
                    [CIN, BPC, R + 2, NCOMP, J], dt.bfloat16, tag=f"v{ht}", name="v"
                )
                vt[ht] = t_
                for n in range(BPC):
                    nc.sync.dma_start(t_[:, n], v_d[ht, :, n])

            # Output staging: per (c, image-pair) buffer holding up to TWO
            # row-tiles (16 rows) in fp16, flushed as one DMA per image with
            # 1792B per-partition descriptors. The odd row-tile (NT=7) is
            # flushed mid-run at ht=4 so the kernel-tail flush keeps the big
            # descriptors: segments (0,1), (2,3), (4), (5,6).
            SEG = {0: (0, 2), 1: (0, 2), 2: (2, 2), 3: (2, 2),
                   4: (4, 1), 5: (5, 2), 6: (5, 2)}
            ot_buf = {}

            for ht in range(NT):
                for c in range(NCH):
                    for p in range(NP):
                        ps = [
                            psum_pool.tile(
                                [128, 2, R, J], dt.float32, tag="ps", name="ps"
                            )
                            for _ in range(NCOMP)
                        ]
                        def mm_group(comp):
                            for kh in range(KH):
                                nc.tensor.matmul(
                                    ps[comp][:],
                                    w_t[:, c, comp, kh],
                                    vt[ht][
                                        :, 2 * p : 2 * p + 2, kh : kh + R, comp
                                    ],
                                    start=(kh == 0),
                                    stop=(kh == KH - 1),
                                )

                        # Inverse transform + bias in 6 ops (scalar 2, vector 2,
                        # gpsimd 2); every op reads at most one PSUM operand and
                        # gpsimd (no PSUM access) gets the SBUF-only finals:
                        #   y_even = ((m1 + b) + m0) + m2
                        #   y_odd  = ((m1 + b) - m3) - m2
                        # Drains are interleaved between accumulation groups so
                        # PSUM banks recycle early and the consumer engines
                        # start mid-group instead of at group end.
                        mm_group(0)
                        mm_group(1)
                        s1b = tmp_pool.tile([128, 2, R, J], dt.float32, tag="s1b")
                        nc.scalar.activation(
                            s1b[:],
                            ps[1][:],
                            mybir.ActivationFunctionType.Identity,
                            bias=b_t[:, c : c + 1],
                        )
                        mm_group(2)
                        s2 = tmp_pool.tile([128, 2, R, J], dt.float32, tag="s2")
                        nc.scalar.activation(
                            s2[:], ps[2][:], mybir.ActivationFunctionType.Identity
                        )
                        ae = tmp_pool.tile([128, 2, R, J], dt.float32, tag="ae")
                        nc.vector.tensor_add(ae[:], s1b[:], ps[0][:])
                        mm_group(3)
                        ao = tmp_pool.tile([128, 2, R, J], dt.float32, tag="ao")
                        nc.vector.tensor_sub(ao[:], s1b[:], ps[3][:])
                        # gpsimd cannot access PSUM: it gets the SBUF-only finals
                        seg0, seglen = SEG[ht]
                        if ht == seg0:
                            ot_buf[(c, p)] = out_pool.tile(
                                [128, 2, 2, R, W_], dt.float16, tag="ot", name="ot"
                            )
                        ot = ot_buf[(c, p)]
                        par = ht - seg0
                        nc.gpsimd.tensor_add(ot[:, :, par, :, 0::2], ae[:], s2[:])
                        nc.gpsimd.tensor_sub(ot[:, :, par, :, 1::2], ao[:], s2[:])

                        if par == seglen - 1:
                            h0 = seg0 * R
                            nrows = seglen * R
                            for i in range(2):
                                nc.sync.dma_start(
                                    o_d[
                                        2 * p + i,
                                        c * 128 : (c + 1) * 128,
                                        h0 : h0 + nrows,
                                        :,
                                    ],
                                    ot[:, i, : par + 1],
                                )

    nc.compile()
    return nc


def _make_in_maps(x, W, b):
    import ml_dtypes

    bf16 = ml_dtypes.bfloat16
    x = np.asarray(x, dtype=np.float32)
    W = np.asarray(W, dtype=np.float32)
    b = np.asarray(b, dtype=np.float32)

    # Pad, then 1D Winograd F(2,3) input transform along W (on padded cols):
    # output pair j uses padded cols 2j..2j+3.
    xpad = np.zeros((B, CIN, HP, W_ + 2), dtype=np.float32)
    xpad[:, :, 1 : H + 1, 1 : W_ + 1] = x
    e = xpad[..., 0::2]  # even padded cols 0,2,..,56 (29)
    o = xpad[..., 1::2]  # odd padded cols 1,3,..,57 (29)
    V = np.empty((B, CIN, HP, NCOMP, J), dtype=np.float32)
    V[:, :, :, 0] = e[..., :J] - e[..., 1 : J + 1]   # d0-d2
    V[:, :, :, 1] = o[..., :J] + e[..., 1 : J + 1]   # d1+d2
    V[:, :, :, 2] = e[..., 1 : J + 1] - o[..., :J]   # d2-d1
    V[:, :, :, 3] = o[..., :J] - o[..., 1 : J + 1]   # d1-d3

    # Re-tile: [B, CIN, HP, 4, J] -> [NT, CIN, B, R+2, 4, J]
    vtiles = np.empty((NT, CIN, B, R + 2, NCOMP, J), dtype=bf16)
    for ht in range(NT):
        vtiles[ht] = V[:, :, ht * R : ht * R + R + 2].transpose(1, 0, 2, 3, 4)

    # Weight transform: per kh tap, comps [w0, (w0+w1+w2)/2, (w0-w1+w2)/2, w2]
    w0, w1, w2 = W[..., 0], W[..., 1], W[..., 2]  # each [COUT, CIN, KH]
    g = np.stack(
        [w0, (w0 + w1 + w2) * 0.5, (w0 - w1 + w2) * 0.5, w2], axis=-1
    )  # [COUT, CIN, KH, 4]
    # -> [chunk, comp, cin, kh, cout_slice]
    wt = np.ascontiguousarray(
        g.reshape(NCH, 128, CIN, KH, NCOMP).transpose(0, 4, 2, 3, 1), dtype=bf16
    )
    bh = np.ascontiguousarray(b.reshape(NCH, 128).T)

    return [
        {
            "v": np.ascontiguousarray(vtiles[:, :, core * BPC : (core + 1) * BPC]),
            "wt": wt,
            "bias": bh,
        }
        for core in range(NCORES)
    ]


def kernel(x, W, b):
    from concourse.bass_utils import run_bass_kernel_spmd

    if "nc" not in _cache:
        _cache["nc"] = _build()
    nc = _cache["nc"]

    in_maps = _make_in_maps(x, W, b)
    try:
        res = run_bass_kernel_spmd(nc, in_maps, list(range(NCORES))).results
    except Exception:
        # A prior session can leave the accelerator in a transient
        # unrecoverable state; one retry after re-init clears it.
        import time

        time.sleep(15)
        res = run_bass_kernel_spmd(nc, in_maps, list(range(NCORES))).results
    return np.concatenate(
        [res[i]["out"].astype(np.float32) for i in range(NCORES)], axis=0
    )


# revision 47
# speedup vs baseline: 1.0323x; 1.0176x over previous
"""Trainium2 Bass kernel for nn_Conv2d: x[32,128,56,56] * W[256,128,3,3] + b -> [32,256,56,56].

Stride 1, padding 1, dilation 1. Data-parallel over batch across 8 NeuronCores
(4 images per core, no collectives).

Per core: 1D Winograd F(2,3) along W. The host transforms the padded input
into 4 components per output-column pair (v0=d0-d2, v1=d1+d2, v2=d2-d1,
v3=d1-d3) and the weights into matching components per vertical tap
(g0=w0, g1=(w0+w1+w2)/2, g2=(w0-w1+w2)/2, g3=w2). On device, each
(row_tile, cout_chunk, image_pair) group runs 12 bf16 matmuls (4 components
x 3 vertical taps, accumulated over taps) into 4 PSUM banks of
[128cout, 2img*8rows*28pairs=448], i.e. 12*448 PE rows per 896 outputs vs
18*448 for direct conv -- a 1.5x tensor-engine reduction. The inverse
transform + bias (y_even = m0+m1+m2+b, y_odd = m1-m2-m3+b) is split across
the scalar (activation with bias/scale), vector, and gpsimd engines and
writes the interleaved output columns, fully hidden under the matmuls.

Self-contained: hardcodes shapes; host does padding/Winograd/bf16 prep so
every device DMA is contiguous.
"""

import numpy as np

B, CIN, H, W_ = 32, 128, 56, 56
COUT, KH, KW = 256, 3, 3
NCORES = 8
BPC = B // NCORES          # images per core
R = 8                      # output rows per tile
NT = H // R                # row tiles per image
HP = H + 2                 # padded rows
J = W_ // 2                # output column pairs
NCOMP = 4                  # Winograd F(2,3) components
NCH = COUT // 128          # cout chunks
NP = BPC // 2              # image pairs per group (matmul free dim 2*R*J=448)

_cache = {}


def _build():
    import concourse.mybir as mybir
    import concourse.tile as tile
    from concourse import bacc

    dt = mybir.dt

    nc = bacc.Bacc("TRN2", target_bir_lowering=False, debug=False)

    # Host-transformed input per row-tile: padded rows ht*R..ht*R+R+1,
    # 4 Winograd components x 28 column pairs. Halo rows are duplicated
    # host-side so every row-tile's DMA is self-contained (a halo-free
    # layout saved 1.4MB but its chunk dependencies stalled the early
    # groups behind ~2.4MB of queued input -- measured slower).
    v_d = nc.dram_tensor(
        "v", [NT, CIN, BPC, R + 2, NCOMP, J], dt.bfloat16, kind="ExternalInput"
    )
    # Host-transformed weights: [chunk, comp, cin, kh, cout_slice] -- comp-major
    # so the first accumulation group's weights arrive in one small DMA.
    wt_d = nc.dram_tensor(
        "wt", [NCH, NCOMP, CIN, KH, 128], dt.bfloat16, kind="ExternalInput"
    )
    b_d = nc.dram_tensor("bias", [128, NCH], dt.float32, kind="ExternalInput")
    # fp16 output halves the dominant DMA term (12.8MB -> 6.4MB per core);
    # the host converts back to fp32. Quantization adds ~5e-4 absmax error.
    o_d = nc.dram_tensor("out", [BPC, COUT, H, W_], dt.float16, kind="ExternalOutput")

    with tile.TileContext(nc) as tc:
        with (
            tc.tile_pool(name="const", bufs=1) as const_pool,
            tc.tile_pool(name="vin", bufs=1) as vin_pool,
            tc.tile_pool(name="tmp", bufs=8) as tmp_pool,
            tc.tile_pool(name="outp", bufs=8) as out_pool,
            tc.tile_pool(name="psum", bufs=8, space="PSUM") as psum_pool,
        ):
            # All NT row-tiles stay resident (~63KB/partition at bf16).
            vt = {}

            # Issue order tracks the first group's critical path: weights
            # chunk 0 and row-tile 0 first, then the rest of the inputs.
            # The Sync engine issues DMA_DIRECT2D at only ~600ns each, so a
            # single issuer serializes the startup stream (~8us before the
            # first v image even starts). Fan the startup-critical issues out
            # over scalar/vector/gpsimd -- idle until the first drain at
            # ~16us -- and leave the long tail (and all output) on sync.
            issuers = [nc.scalar, nc.gpsimd, nc.sync]
            k = [0]

            def early_dma(dst, src):
                issuers[k[0] % 3].dma_start(dst, src)
                k[0] += 1

            w_t = const_pool.tile([CIN, NCH, NCOMP, KH, 128], dt.bfloat16)
            t0_ = vin_pool.tile([CIN, BPC, R + 2, NCOMP, J], dt.bfloat16, tag="v0")
            vt[0] = t0_
            early_dma(w_t[:, 0, 0], wt_d[0, 0])
            early_dma(t0_[:, 0], v_d[0, :, 0])
            early_dma(t0_[:, 1], v_d[0, :, 1])
            for comp in range(1, NCOMP):
                early_dma(w_t[:, 0, comp], wt_d[0, comp])
            early_dma(t0_[:, 2], v_d[0, :, 2])
            early_dma(t0_[:, 3], v_d[0, :, 3])
            for comp in range(NCOMP):
                early_dma(w_t[:, 1, comp], wt_d[1, comp])
            b_t = const_pool.tile([128, NCH], dt.float32)
            nc.sync.dma_start(b_t[:], b_d[:])
            for ht in range(1, NT):
                t_ = vin_pool.tile(
                    [CIN, BPC, R + 2, NCOMP, J], dt.bfloat16, tag=f"v{ht}", name="v"
                )
                vt[ht] = t_
                for n in range(BPC):
                    nc.sync.dma_start(t_[:, n], v_d[ht, :, n])

            # Output staging: per (c, image-pair) buffer holding up to TWO
            # row-tiles (16 rows) in fp16, flushed as one DMA per image with
            # 1792B per-partition descriptors. The odd row-tile (NT=7) is
            # flushed mid-run at ht=4 so the kernel-tail flush keeps the big
            # descriptors: segments (0,1), (2,3), (4), (5,6).
            SEG = {0: (0, 2), 1: (0, 2), 2: (2, 2), 3: (2, 2),
                   4: (4, 1), 5: (5, 2), 6: (5, 2)}
            ot_buf = {}

            for ht in range(NT):
                for c in range(NCH):
                    for p in range(NP):
                        ps = [
                            psum_pool.tile(
                                [128, 2, R, J], dt.float32, tag="ps", name="ps"
                            )
                            for _ in range(NCOMP)
                        ]
                        def mm_group(comp):
                            for kh in range(KH):
                                nc.tensor.matmul(
                                    ps[comp][:],
                                    w_t[:, c, comp, kh],
                                    vt[ht][
                                        :, 2 * p : 2 * p + 2, kh : kh + R, comp
                                    ],
                                    start=(kh == 0),
                                    stop=(kh == KH - 1),
                                )

                        # Inverse transform + bias in 6 ops (scalar 2, vector 2,
                        # gpsimd 2); every op reads at most one PSUM operand and
                        # gpsimd (no PSUM access) gets the SBUF-only finals:
                        #   y_even = ((m1 + b) + m0) + m2
                        #   y_odd  = ((m1 + b) - m3) - m2
                        # Drains are interleaved between accumulation groups so
                        # PSUM banks recycle early and the consumer engines
                        # start mid-group instead of at group end.
                        mm_group(0)
                        mm_group(1)
                        s1b = tmp_pool.tile([128, 2, R, J], dt.float32, tag="s1b")
                        nc.scalar.activation(
                            s1b[:],
                            ps[1][:],
                            mybir.ActivationFunctionType.Identity,
                            bias=b_t[:, c : c + 1],
                        )
                        mm_group(2)
                        s2 = tmp_pool.tile([128, 2, R, J], dt.float32, tag="s2")
                        nc.scalar.activation(
                            s2[:], ps[2][:], mybir.ActivationFunctionType.Identity
                        )
                        ae = tmp_pool.tile([128, 2, R, J], dt.float32, tag="ae")
                        nc.vector.tensor_add(ae[:], s1b[:], ps[0][:])
                        mm_group(3)
                        ao = tmp_pool.tile([128, 2, R, J], dt.float32, tag="ao")
                        nc.vector.tensor_sub(ao[:], s1b[:], ps[3][:])
                        # gpsimd cannot access PSUM: it gets the SBUF-only finals
                        seg0, seglen = SEG[ht]
                        if ht == seg0:
                            ot_buf[(c, p)] = out_pool.tile(
                                [128, 2, 2, R, W_], dt.float16, tag="ot", name="ot"
                            )
                        ot = ot_buf[(c, p)]
                        par = ht - seg0
                        nc.gpsimd.tensor_add(ot[:, :, par, :, 0::2], ae[:], s2[:])
                        nc.gpsimd.tensor_sub(ot[:, :, par, :, 1::2], ao[:], s2[:])

                        if par == seglen - 1:
                            h0 = seg0 * R
                            nrows = seglen * R
                            for i in range(2):
                                nc.sync.dma_start(
                                    o_d[
                                        2 * p + i,
                                        c * 128 : (c + 1) * 128,
                                        h0 : h0 + nrows,
                                        :,
                                    ],
                                    ot[:, i, : par + 1],
                                )

    nc.compile()
    return nc


def _make_in_maps(x, W, b):
    import ml_dtypes

    bf16 = ml_dtypes.bfloat16
    x = np.asarray(x, dtype=np.float32)
    W = np.asarray(W, dtype=np.float32)
    b = np.asarray(b, dtype=np.float32)

    # Pad, then 1D Winograd F(2,3) input transform along W (on padded cols):
    # output pair j uses padded cols 2j..2j+3.
    xpad = np.zeros((B, CIN, HP, W_ + 2), dtype=np.float32)
    xpad[:, :, 1 : H + 1, 1 : W_ + 1] = x
    e = xpad[..., 0::2]  # even padded cols 0,2,..,56 (29)
    o = xpad[..., 1::2]  # odd padded cols 1,3,..,57 (29)
    V = np.empty((B, CIN, HP, NCOMP, J), dtype=np.float32)
    V[:, :, :, 0] = e[..., :J] - e[..., 1 : J + 1]   # d0-d2
    V[:, :, :, 1] = o[..., :J] + e[..., 1 : J + 1]   # d1+d2
    V[:, :, :, 2] = e[..., 1 : J + 1] - o[..., :J]   # d2-d1
    V[:, :, :, 3] = o[..., :J] - o[..., 1 : J + 1]   # d1-d3

    # Re-tile: [B, CIN, HP, 4, J] -> [NT, CIN, B, R+2, 4, J]
    vtiles = np.empty((NT, CIN, B, R + 2, NCOMP, J), dtype=bf16)
    for ht in range(NT):
        vtiles[ht] = V[:, :, ht * R : ht * R + R + 2].transpose(1, 0, 2, 3, 4)

    # Weight transform: per kh tap, comps [w0, (w0+w1+w2)/2, (w0-w1+w2)/2, w2]
    w0, w1, w2 = W[..., 0], W[..., 1], W[..., 2]  # each [COUT, CIN, KH]
    g = np.stack(
        [w0, (w0 + w1 + w2) * 0.5, (w0 - w1 + w2) * 0.5, w2], axis=-1
    )  # [COUT, CIN, KH, 4]
    # -> [chunk, comp, cin, kh, cout_slice]
    wt = np.ascontiguousarray(
        g.reshape(NCH, 128, CIN, KH, NCOMP).transpose(0, 4, 2, 3, 1), dtype=bf16
    )
    bh = np.ascontiguousarray(b.reshape(NCH, 128).T)

    return [
        {
            "v": np.ascontiguousarray(vtiles[:, :, core * BPC : (core + 1) * BPC]),
            "wt": wt,
            "bias": bh,
        }
        for core in range(NCORES)
    ]


def kernel(x, W, b):
    from concourse.bass_utils import run_bass_kernel_spmd

    if "nc" not in _cache:
        _cache["nc"] = _build()
    nc = _cache["nc"]

    in_maps = _make_in_maps(x, W, b)
    try:
        res = run_bass_kernel_spmd(nc, in_maps, list(range(NCORES))).results
    except Exception:
        # A prior session can leave the accelerator in a transient
        # unrecoverable state; one retry after re-init clears it.
        import time

        time.sleep(15)
        res = run_bass_kernel_spmd(nc, in_maps, list(range(NCORES))).results
    return np.concatenate(
        [res[i]["out"].astype(np.float32) for i in range(NCORES)], axis=0
    )
